# revision 44
# baseline (speedup 1.0000x reference)
"""Trainium2 Bass kernel for BertAlibiUnpadSelfAttention.

Problem shapes (hardcoded): B=2, S=2048, H=12, D=64, DIM=768.
Reference computation:
    qkv = hidden @ Wqkv_w.T + Wqkv_b            # (4096, 2304)
    pad via indices (a permutation -> pure row shuffle)
    q,k,v = split/reshape -> (b, h, s, d)
    scores = q @ k.T / sqrt(64) + bias          # bias dense (2,12,2048,2048)
    attn = softmax(scores) @ v -> (4096, 768), unpad via indices

Sharding: 24 (batch, head) pairs -> 3 per core across 8 cores.

v2 layout: single fused PE stream.  The attention loop is a 3-engine
software pipeline (PE scores -> ACT exp -> DVE mul -> PE PV) with the PV
lagging 2 blocks behind its QK so the PE never waits on the exp chain.
The q/k projection is interleaved into the attention stream as PE filler
work; only the V projection and the first q0/k0 chunks run up-front.
Blocks >= INJ_FROM add the bias in PSUM via an fp16 identity matmul
(denser PE stream, keeps HAM warm) instead of the DVE multiply.
"""

import math
import numpy as np

B, S, H, D = 2, 2048, 12, 64
DIM = H * D            # 768
TOTAL = B * S          # 4096
HPC = 3                # heads per core
N_CORES = 8
KT = DIM // 128        # 6 k-tiles of 128
SKT = S // 128         # 16 sk tiles of 128
NBLK = HPC * 2 * SKT   # 96 attention blocks per core

_CACHE = {}


def _inj(b, inj_from):
    return b >= inj_from


def _build_nc(inj_from=64):
    from concourse import bacc, mybir, tile

    f32 = mybir.dt.float32
    f16 = mybir.dt.float16

    nc = bacc.Bacc("TRN2", target_bir_lowering=False, debug=False)

    hT = nc.dram_tensor("hT", (DIM, S), f16, kind="ExternalInput")
    # host-packed critical bundle: [wk tiles | wq tiles | hT cols 0-511],
    # one DMA so the first projection chunks unblock ASAP
    early = nc.dram_tensor("early", (128, 2 * KT * HPC * D + KT * 512), f16,
                           kind="ExternalInput")
    # host-packed wv tiles [p, i*192+c] = Wv.T[i*128+p, c], followed by
    # head-2 [wq1|wk1] packed tiles for the m=128 combined q1/k1 chunks
    wvp = nc.dram_tensor("wvp", (128, KT * HPC * D + KT * 128), f16,
                         kind="ExternalInput")
    # host-packed projection biases:
    # cols = [bq lo, bq hi, bk lo, bk hi, bq1;bk1 stacked]
    bvec = nc.dram_tensor("bvec", (128, 5), f32, kind="ExternalInput")
    # (j, sqh, c4, 128, 4096): host-rearranged bias; exp'd for expb blocks,
    # raw for inject blocks.
    bias_r = nc.dram_tensor("bias_r", (HPC, 2, 4, 128, 4096), f16,
                            kind="ExternalInput")
    ident = nc.dram_tensor("ident", (128, 128), f16, kind="ExternalInput")
    out = nc.dram_tensor("out", (HPC, 2, D + 1, 1024), f32,
                         kind="ExternalOutput")

    EXP = mybir.ActivationFunctionType.Exp

    with tile.TileContext(nc) as tc:
        with (
            tc.tile_pool(name="const", bufs=1) as constp,
            tc.tile_pool(name="bias", bufs=4) as biasp,
            tc.tile_pool(name="pt", bufs=5) as ptp,
            tc.tile_pool(name="ot", bufs=2) as otp,
        ):
            # ---- persistent SBUF tiles ----
            ht = [constp.tile([128, S], f16, tag=f"ht{i}", name=f"ht{i}")
                  for i in range(KT)]
            WB = HPC * D  # 192
            early_sb = constp.tile([128, 2 * KT * WB + KT * 512], f16, tag="early")
            wk_sb = [early_sb[:, i * WB:(i + 1) * WB] for i in range(KT)]
            wq_sb = [early_sb[:, (KT + i) * WB:(KT + i + 1) * WB] for i in range(KT)]
            wvp_sb = constp.tile([128, KT * WB + KT * 128], f16, tag="wvp")
            wv_sb = [wvp_sb[:, i * WB:(i + 1) * WB] for i in range(KT)]
            wqk1_sb = [wvp_sb[:, KT * WB + i * 128:KT * WB + (i + 1) * 128]
                       for i in range(KT)]
            HT0 = 2 * KT * WB  # offset of the hT cols 0-511 block in early_sb

            def ht_src(i, lo, hi):
                """hT[i][:, lo:hi]; cols 0-511 live in the early bundle."""
                if hi <= 512:
                    return early_sb[:, HT0 + i * 512 + lo:HT0 + i * 512 + hi]
                assert lo >= 512
                return ht[i][:, lo:hi]
            bvec_sb = constp.tile([128, 5], f32, tag="bvec")
            bq_sb = bvec_sb[:, 0:1]
            bk_sb = bvec_sb[:, 2:3]
            bqk1_sb = bvec_sb[:, 4:5]   # [bq1(64) ; bk1(64)] stacked
            ident_sb = constp.tile([128, 128], f16, tag="ident")
            # q/k in [d, s] layout; heads 0,1 stacked in q0/k0 partitions.
            # Head 2: qk1 holds [q1; k1] stacked (proj as m=128 chunks); k1
            # is then realigned to partitions 0-63 by an SBUF->SBUF DMA.
            q0 = constp.tile([128, S], f16, tag="q0")
            k0 = constp.tile([128, S], f16, tag="k0")
            qk1 = constp.tile([128, S], f16, tag="qk1")
            k1a = constp.tile([64, S], f16, tag="k1a")
            # V' per head: [sk, 65] blocks along free dim; col 64 stays 1.0
            # so PV also produces the softmax row-sums.
            vp = [constp.tile([128, SKT * (D + 1)], f16, tag=f"vp{j}", name=f"vp{j}")
                  for j in range(HPC)]

            # ---- input DMAs ride the scalar HWDGE ring (ACT queue is idle
            # until the first exp anyway); bias gets the sync ring from t=0.
            # Critical-path order: wk+wq tiles, then hT in 512-column pieces
            # so the first proj chunks unblock ASAP ----
            wsrc = constp.tile([64, 512], f16, tag="wsrc")
            nc.vector.memset(wsrc[:], 0.0)
            nc.scalar.dma_start(early_sb[:], early[:, :])
            nc.scalar.dma_start(wvp_sb[:], wvp[:, :])
            nc.scalar.dma_start(bvec_sb[:], bvec[:, :])
            for c in range(1, 4):
                cs = slice(c * 512, (c + 1) * 512)
                for i in range(KT):
                    nc.scalar.dma_start(ht[i][:, cs], hT[i * 128:(i + 1) * 128, cs])
                if c == 1:
                    nc.scalar.dma_start(ident_sb[:], ident[:, :])
            for j in range(HPC):
                nc.vector.memset(vp[j][:], 1.0)

            # ---- projection chunk helpers ----
            # q/k chunks: psum [m, 512] accumulated over KT, bias-added into
            # the persistent qT/kT tiles by DVE.
            def qk_chunk(pool, dst, wsb, bsb, col0, m, c, tag="pk", bufs=2,
                         mm_per_unit=KT):
                """Projection chunk (6 matmuls + bias-add) as a list of unit
                callables emitting mm_per_unit matmuls each; the PSUM slot is
                held from the first unit to the bias-add on the last."""
                state = {}

                def unit(i0):
                    def emit():
                        if i0 == 0:
                            state["ps"] = pool.tile([m, 512], f32, tag=tag,
                                                    name=f"pk{m}", bufs=bufs)
                        ps = state["ps"]
                        for i in range(i0, i0 + mm_per_unit):
                            nc.tensor.matmul(
                                ps[:],
                                wsb[i][:, col0:col0 + m],
                                ht_src(i, c * 512, (c + 1) * 512),
                                start=(i == 0), stop=(i == KT - 1),
                            )
                        if i0 + mm_per_unit == KT:
                            nc.vector.tensor_scalar_add(
                                dst[:, c * 512:(c + 1) * 512], ps[:], bsb[:])
                    return emit
                units = [unit(i0) for i0 in range(0, KT, mm_per_unit)]
                if mm_per_unit == KT:
                    return units[0]
                return units

            def v_tile(pool, st, tag="pv", bufs=2):
                def emit():
                    psv = pool.tile([128, HPC * D], f32, tag=tag, name="psv",
                                    bufs=bufs)
                    for i in range(KT):
                        nc.tensor.matmul(
                            psv[:],
                            ht_src(i, st * 128, (st + 1) * 128),
                            wv_sb[i][:],
                            start=(i == 0), stop=(i == KT - 1),
                        )
                    for j in range(HPC):
                        nc.vector.tensor_copy(
                            vp[j][:, st * (D + 1):st * (D + 1) + D],
                            psv[:, j * D:(j + 1) * D])
                return emit

            # ---- upfront: HAM pre-warm + k0/q0 c0 + q0 c1 + first V tiles ----
            with tc.tile_pool(name="projA", bufs=2, space="PSUM") as projp:
                # one long accumulation group: back-to-back, no WAW sems
                warm_ps = projp.tile([64, 512], f32, tag="warm", bufs=1)
                NWARM = 18
                for i in range(NWARM):
                    nc.tensor.matmul(warm_ps[:], wsrc[:, 0:64], wsrc[:],
                                     start=(i == 0), stop=(i == NWARM - 1))
                nc.vector.tensor_copy(wsrc[:, 0:64], warm_ps[:, 0:64])
                qk_chunk(projp, k0, wk_sb, bk_sb, 0, 128, 0)()
                qk_chunk(projp, q0, wq_sb, bq_sb, 0, 128, 0)()
                qk_chunk(projp, q0, wq_sb, bq_sb, 0, 128, 1)()
                for st in range(6):
                    v_tile(projp, st)()

            # ---- attention: 96-block pipeline with interleaved proj ----
            qk_slices = (
                (q0[0:64, :], k0[0:64, :]),
                (q0[64:128, :], k0[64:128, :]),
                (qk1[0:64, :], k1a[:, :]),
            )

            with tc.tile_pool(name="att", bufs=3, space="PSUM") as attp:
                # filler schedule: block -> one full proj unit (a chunk or a
                # V tile), borrowing a "ps"-tag PSUM slot for its lifetime
                filler = {b: [] for b in range(NBLK)}

                def aux_chunk(dst, wsb, bsb, col0, m, c):
                    return qk_chunk(attp, dst, wsb, bsb, col0, m, c,
                                    tag="ps", bufs=3)

                filler[0].append(aux_chunk(k0, wk_sb, bk_sb, 0, 128, 1))
                filler[3].append(aux_chunk(k0, wk_sb, bk_sb, 0, 128, 2))
                filler[6].append(aux_chunk(k0, wk_sb, bk_sb, 0, 128, 3))
                filler[10].append(aux_chunk(q0, wq_sb, bq_sb, 0, 128, 2))
                filler[13].append(aux_chunk(q0, wq_sb, bq_sb, 0, 128, 3))
                vslots = [1, 2, 4, 5, 7, 8, 9, 11, 12, 14]
                for st, b in zip(range(6, SKT), vslots):
                    filler[b].append(v_tile(attp, st, tag="ps", bufs=3))
                # head-2 [q1;k1] m=128 chunks, then realign k1 to partitions
                # 0-63 with an SBUF->SBUF DMA (off-engine)
                for c in range(4):
                    filler[17 + 6 * c].append(
                        aux_chunk(qk1, wqk1_sb, bqk1_sb, 0, 128, c))
                filler[44].append(
                    lambda: nc.sync.dma_start(k1a[:], qk1[64:128, :]))

                blocks = [(j, sqh, st)
                          for j in range(HPC) for sqh in range(2)
                          for st in range(SKT)]
                pend = []            # PV lag queue: (j, sqh, st, pt, po)
                po_cur = {}
                bt_cur = {}

                def emit_pv(j, sqh, st, pt, po):
                    last = st == SKT - 1
                    ot = otp.tile([D + 1, 1024], f32, name="ot") if last else None
                    for cc in range(2):
                        nc.tensor.matmul(
                            po[:, cc * 512:(cc + 1) * 512],
                            vp[j][:, st * (D + 1):(st + 1) * (D + 1)],
                            pt[:, cc * 512:(cc + 1) * 512],
                            start=(st == 0), stop=last,
                        )
                        if last:
                            nc.vector.tensor_copy(
                                ot[:, cc * 512:(cc + 1) * 512],
                                po[:, cc * 512:(cc + 1) * 512])
                    if last:
                        nc.sync.dma_start(out[j, sqh], ot[:])

                for b, (j, sqh, st) in enumerate(blocks):
                    if st % 4 == 0:
                        bt = biasp.tile([128, 4096], f16, name="bt")
                        nc.sync.dma_start(bt[:], bias_r[j, sqh, st // 4])
                        bt_cur[(j, sqh)] = bt
                    bt = bt_cur[(j, sqh)]
                    boff = (st % 4) * 1024
                    if st == 0:
                        po_cur[(j, sqh)] = attp.tile([D + 1, 1024], f32,
                                                     tag="po", name="po", bufs=1)
                    qap, kap = qk_slices[j]
                    inj = _inj(b, inj_from)
                    ps = attp.tile([128, 1024], f32, tag="ps", name="ps", bufs=3)
                    if inj:
                        for cc in range(2):
                            nc.tensor.matmul(
                                ps[:, cc * 512:(cc + 1) * 512],
                                ident_sb[:],
                                bt[:, boff + cc * 512:boff + (cc + 1) * 512],
                                start=True, stop=False,
                            )
                    for cc in range(2):
                        nc.tensor.matmul(
                            ps[:, cc * 512:(cc + 1) * 512],
                            kap[:, st * 128:(st + 1) * 128],
                            qap[:, sqh * 1024 + cc * 512:sqh * 1024 + (cc + 1) * 512],
                            start=(not inj), stop=True,
                        )
                    for f in filler[b]:
                        f()
                    pt = ptp.tile([128, 1024], f16, name="pt")
                    nc.scalar.activation(pt[:], ps[:], EXP)
                    if not inj:
                        nc.vector.tensor_mul(pt[:], pt[:],
                                             bt[:, boff:boff + 1024])
                    pend.append((j, sqh, st, pt, po_cur[(j, sqh)]))
                    if len(pend) > 3:
                        emit_pv(*pend.pop(0))
                while pend:
                    emit_pv(*pend.pop(0))

    nc.compile()
    return nc


def _get_nc(inj_from=64):
    if inj_from not in _CACHE:
        _CACHE[inj_from] = _build_nc(inj_from)
    return _CACHE[inj_from]


def _make_in_maps(hidden_states, Wqkv_w, Wqkv_b, bias, indices, inj_from=64):
    hidden_states = np.asarray(hidden_states, dtype=np.float32)
    Wqkv_w = np.asarray(Wqkv_w, dtype=np.float32)
    Wqkv_b = np.asarray(Wqkv_b, dtype=np.float32)
    bias = np.asarray(bias, dtype=np.float32)
    indices = np.asarray(indices, dtype=np.int64)

    scale = 1.0 / math.sqrt(D)
    padded = np.zeros((TOTAL, DIM), dtype=np.float32)
    padded[indices] = hidden_states

    Wq, Wk, Wv = Wqkv_w[0:DIM], Wqkv_w[DIM:2 * DIM], Wqkv_w[2 * DIM:3 * DIM]
    bq_full = Wqkv_b[0:DIM] * scale
    bk_full = Wqkv_b[DIM:2 * DIM]
    ident = np.eye(128, dtype=np.float16)

    in_maps = []
    for c in range(N_CORES):
        b = c // 4
        h0 = (c % 4) * HPC
        r = slice(h0 * D, (h0 + HPC) * D)
        # bias_r[j, sqh, c4, p, st4*1024 + q] =
        #   f(bias[b, h0+j, sqh*1024 + q, (4*c4+st4)*128 + p])   (transposed)
        bias_c = bias[b, h0:h0 + HPC]                    # (3, sq, sk)
        bt = bias_c.transpose(0, 2, 1)                   # (3, sk, sq)
        bt = bt.reshape(HPC, 4, 4, 128, 2, 1024)         # (j, c4, st4, p, sqh, q)
        bt = bt.transpose(0, 4, 1, 3, 2, 5)              # (j, sqh, c4, p, st4, q)
        blk = (np.arange(HPC)[:, None, None, None] * 2 * SKT
               + np.arange(2)[None, :, None, None] * SKT
               + np.arange(4)[None, None, :, None] * 4
               + np.arange(4)[None, None, None, :])      # (j, sqh, c4, st4)
        expb = blk < inj_from
        bt = np.where(expb[:, :, :, None, :, None], np.exp(bt), bt)
        bias_r = np.ascontiguousarray(
            bt.reshape(HPC, 2, 4, 128, 4096).astype(np.float16))
        # pack k/q weight k-tiles + hT cols 0-511 into the "early" bundle:
        # early[p, (w*6+i)*192+c] = W.T[i*128+p, c];
        # early[p, 2304+i*512+c]  = hT[i*128+p, c]  (c < 512)
        hTc = padded[b * S:(b + 1) * S].T.astype(np.float16)  # (768, 2048)
        wt = np.stack([Wk[r].T, (Wq[r].T * np.float32(scale))])
        wkq = (wt.reshape(2, KT, 128, HPC * D).transpose(2, 0, 1, 3)
               .reshape(128, 2 * KT * HPC * D).astype(np.float16))
        ht0 = (hTc[:, 0:512].reshape(KT, 128, 512).transpose(1, 0, 2)
               .reshape(128, KT * 512))
        early = np.ascontiguousarray(np.concatenate([wkq, ht0], axis=1))
        wv_t = (Wv[r].T.reshape(KT, 128, HPC * D).transpose(1, 0, 2)
                .reshape(128, KT * HPC * D).astype(np.float16))
        # head-2 packed tiles: [wq1(64) | wk1(64)] per k-tile
        wq1 = (Wq[r].T * np.float32(scale))[:, 128:192]
        wk1 = Wk[r].T[:, 128:192]
        wqk1 = (np.concatenate([wq1, wk1], axis=1).astype(np.float16)
                .reshape(KT, 128, 128).transpose(1, 0, 2).reshape(128, KT * 128))
        wvp = np.ascontiguousarray(np.concatenate([wv_t, wqk1], axis=1))
        bvec = np.zeros((128, 5), dtype=np.float32)
        bvec[:, 0] = bq_full[r][0:128]
        bvec[0:64, 1] = bq_full[r][128:192]
        bvec[:, 2] = bk_full[r][0:128]
        bvec[0:64, 3] = bk_full[r][128:192]
        bvec[0:64, 4] = bq_full[r][128:192]
        bvec[64:128, 4] = bk_full[r][128:192]
        in_maps.append({
            "hT": hTc,
            "early": early,
            "wvp": wvp,
            "bvec": bvec,
            "bias_r": bias_r,
            "ident": ident,
        })
    return in_maps


def _assemble(results, Wqkv_b, indices):
    Wqkv_b = np.asarray(Wqkv_b, dtype=np.float32)
    indices = np.asarray(indices, dtype=np.int64)
    bv = Wqkv_b[2 * DIM:3 * DIM]
    out_full = np.empty((TOTAL, DIM), dtype=np.float32)
    for c in range(N_CORES):
        b = c // 4
        h0 = (c % 4) * HPC
        o = np.asarray(results[c]["out"], dtype=np.float32)  # (3, 2, 65, 1024)
        for j in range(HPC):
            h = h0 + j
            oj = np.concatenate([o[j, 0], o[j, 1]], axis=1)  # (65, 2048)
            att = (oj[:D] / oj[D]).T + bv[h * D:(h + 1) * D]
            out_full[b * S:(b + 1) * S, h * D:(h + 1) * D] = att
    return out_full[indices]


INJ_FROM = 96


def kernel(hidden_states, Wqkv_w, Wqkv_b, bias, slopes, cu_seqlens, indices,
           attn_mask, max_seqlen, **_unused):
    from concourse.bass_utils import run_bass_kernel_spmd

    nc = _get_nc(INJ_FROM)
    in_maps = _make_in_maps(hidden_states, Wqkv_w, Wqkv_b, bias, indices,
                            INJ_FROM)
    res = run_bass_kernel_spmd(nc, in_maps, list(range(N_CORES)))
    return _assemble(res.results, Wqkv_b, indices)


# revision 45
# speedup vs baseline: 1.1898x; 1.1898x over previous
"""Trainium2 Bass kernel for BertAlibiUnpadSelfAttention.

Problem shapes (hardcoded): B=2, S=2048, H=12, D=64, DIM=768.
Reference computation:
    qkv = hidden @ Wqkv_w.T + Wqkv_b            # (4096, 2304)
    pad via indices (a permutation -> pure row shuffle)
    q,k,v = split/reshape -> (b, h, s, d)
    scores = q @ k.T / sqrt(64) + bias          # bias dense (2,12,2048,2048)
    attn = softmax(scores) @ v -> (4096, 768), unpad via indices

Sharding: 24 (batch, head) pairs -> 3 per core across 8 cores.

v2 layout: single fused PE stream.  The attention loop is a 3-engine
software pipeline (PE scores -> ACT exp -> DVE mul -> PE PV) with the PV
lagging 2 blocks behind its QK so the PE never waits on the exp chain.
The q/k projection is interleaved into the attention stream as PE filler
work; only the V projection and the first q0/k0 chunks run up-front.
Blocks >= INJ_FROM add the bias in PSUM via an fp16 identity matmul
(denser PE stream, keeps HAM warm) instead of the DVE multiply.
"""

import math
import numpy as np

B, S, H, D = 2, 2048, 12, 64
DIM = H * D            # 768
TOTAL = B * S          # 4096
HPC = 3                # heads per core
N_CORES = 8
KT = DIM // 128        # 6 k-tiles of 128
SKT = S // 128         # 16 sk tiles of 128
NBLK = HPC * 2 * SKT   # 96 attention blocks per core

_CACHE = {}


def _inj(b, inj_from):
    return b >= inj_from


def _build_nc(inj_from=64):
    from concourse import bacc, mybir, tile

    f32 = mybir.dt.float32
    f16 = mybir.dt.float16

    nc = bacc.Bacc("TRN2", target_bir_lowering=False, debug=False)

    hT = nc.dram_tensor("hT", (DIM, S), f16, kind="ExternalInput")
    # host-packed critical bundle: [wk tiles | wq tiles | hT cols 0-511],
    # one DMA so the first projection chunks unblock ASAP
    early = nc.dram_tensor("early", (128, 2 * KT * HPC * D + KT * 512), f16,
                           kind="ExternalInput")
    # host-packed wv tiles [p, i*192+c] = Wv.T[i*128+p, c], followed by
    # head-2 [wq1|wk1] packed tiles for the m=128 combined q1/k1 chunks
    wvp = nc.dram_tensor("wvp", (128, KT * HPC * D + KT * 128), f16,
                         kind="ExternalInput")
    # host-packed projection biases:
    # cols = [bq lo, bq hi, bk lo, bk hi, bq1;bk1 stacked]
    bvec = nc.dram_tensor("bvec", (128, 5), f32, kind="ExternalInput")
    # (j, sqh, c4, 128, 4096): host-rearranged bias; exp'd for expb blocks,
    # raw for inject blocks.
    bias_r = nc.dram_tensor("bias_r", (HPC, 2, 4, 128, 4096), f16,
                            kind="ExternalInput")
    ident = nc.dram_tensor("ident", (128, 128), f16, kind="ExternalInput")
    out = nc.dram_tensor("out", (HPC, 2, D + 1, 1024), f32,
                         kind="ExternalOutput")

    EXP = mybir.ActivationFunctionType.Exp

    with tile.TileContext(nc) as tc:
        with (
            tc.tile_pool(name="const", bufs=1) as constp,
            tc.tile_pool(name="bias", bufs=4) as biasp,
            tc.tile_pool(name="pt", bufs=5) as ptp,
            tc.tile_pool(name="ot", bufs=2) as otp,
        ):
            # ---- persistent SBUF tiles ----
            ht = [constp.tile([128, S], f16, tag=f"ht{i}", name=f"ht{i}")
                  for i in range(KT)]
            WB = HPC * D  # 192
            early_sb = constp.tile([128, 2 * KT * WB + KT * 512], f16, tag="early")
            wk_sb = [early_sb[:, i * WB:(i + 1) * WB] for i in range(KT)]
            wq_sb = [early_sb[:, (KT + i) * WB:(KT + i + 1) * WB] for i in range(KT)]
            wvp_sb = constp.tile([128, KT * WB + KT * 128], f16, tag="wvp")
            wv_sb = [wvp_sb[:, i * WB:(i + 1) * WB] for i in range(KT)]
            wqk1_sb = [wvp_sb[:, KT * WB + i * 128:KT * WB + (i + 1) * 128]
                       for i in range(KT)]
            HT0 = 2 * KT * WB  # offset of the hT cols 0-511 block in early_sb

            def ht_src(i, lo, hi):
                """hT[i][:, lo:hi]; cols 0-511 live in the early bundle."""
                if hi <= 512:
                    return early_sb[:, HT0 + i * 512 + lo:HT0 + i * 512 + hi]
                assert lo >= 512
                return ht[i][:, lo:hi]
            bvec_sb = constp.tile([128, 5], f32, tag="bvec")
            bq_sb = bvec_sb[:, 0:1]
            bk_sb = bvec_sb[:, 2:3]
            bqk1_sb = bvec_sb[:, 4:5]   # [bq1(64) ; bk1(64)] stacked
            ident_sb = constp.tile([128, 128], f16, tag="ident")
            # q/k in [d, s] layout; heads 0,1 stacked in q0/k0 partitions.
            # Head 2: qk1 holds [q1; k1] stacked (proj as m=128 chunks); k1
            # is then realigned to partitions 0-63 by an SBUF->SBUF DMA.
            q0 = constp.tile([128, S], f16, tag="q0")
            k0 = constp.tile([128, S], f16, tag="k0")
            qk1 = constp.tile([128, S], f16, tag="qk1")
            k1a = constp.tile([64, S], f16, tag="k1a")
            # V' per head: [sk, 65] blocks along free dim; col 64 stays 1.0
            # so PV also produces the softmax row-sums.
            vp = [constp.tile([128, SKT * (D + 1)], f16, tag=f"vp{j}", name=f"vp{j}")
                  for j in range(HPC)]

            # ---- input DMAs ride the scalar HWDGE ring (ACT queue is idle
            # until the first exp anyway); bias gets the sync ring from t=0.
            # Critical-path order: wk+wq tiles, then hT in 512-column pieces
            # so the first proj chunks unblock ASAP ----
            wsrc = constp.tile([64, 512], f16, tag="wsrc")
            nc.vector.memset(wsrc[:], 0.0)
            nc.scalar.dma_start(early_sb[:], early[:, :])
            nc.scalar.dma_start(wvp_sb[:], wvp[:, :])
            nc.scalar.dma_start(bvec_sb[:], bvec[:, :])
            for c in range(1, 4):
                cs = slice(c * 512, (c + 1) * 512)
                for i in range(KT):
                    nc.scalar.dma_start(ht[i][:, cs], hT[i * 128:(i + 1) * 128, cs])
                if c == 1:
                    nc.scalar.dma_start(ident_sb[:], ident[:, :])
            for j in range(HPC):
                nc.vector.memset(vp[j][:], 1.0)

            # ---- projection chunk helpers ----
            # q/k chunks: psum [m, 512] accumulated over KT, bias-added into
            # the persistent qT/kT tiles by DVE.
            def qk_chunk(pool, dst, wsb, bsb, col0, m, c, tag="pk", bufs=2,
                         mm_per_unit=KT):
                """Projection chunk (6 matmuls + bias-add) as a list of unit
                callables emitting mm_per_unit matmuls each; the PSUM slot is
                held from the first unit to the bias-add on the last."""
                state = {}

                def unit(i0):
                    def emit():
                        if i0 == 0:
                            state["ps"] = pool.tile([m, 512], f32, tag=tag,
                                                    name=f"pk{m}", bufs=bufs)
                        ps = state["ps"]
                        for i in range(i0, i0 + mm_per_unit):
                            nc.tensor.matmul(
                                ps[:],
                                wsb[i][:, col0:col0 + m],
                                ht_src(i, c * 512, (c + 1) * 512),
                                start=(i == 0), stop=(i == KT - 1),
                            )
                        if i0 + mm_per_unit == KT:
                            nc.vector.tensor_scalar_add(
                                dst[:, c * 512:(c + 1) * 512], ps[:], bsb[:])
                    return emit
                units = [unit(i0) for i0 in range(0, KT, mm_per_unit)]
                if mm_per_unit == KT:
                    return units[0]
                return units

            def v_tile(pool, st, tag="pv", bufs=2):
                def emit():
                    psv = pool.tile([128, HPC * D], f32, tag=tag, name="psv",
                                    bufs=bufs)
                    for i in range(KT):
                        nc.tensor.matmul(
                            psv[:],
                            ht_src(i, st * 128, (st + 1) * 128),
                            wv_sb[i][:],
                            start=(i == 0), stop=(i == KT - 1),
                        )
                    for j in range(HPC):
                        nc.vector.tensor_copy(
                            vp[j][:, st * (D + 1):st * (D + 1) + D],
                            psv[:, j * D:(j + 1) * D])
                return emit

            # ---- upfront: HAM pre-warm + k0/q0 c0 + q0 c1 + first V tiles ----
            with tc.tile_pool(name="projA", bufs=2, space="PSUM") as projp:
                warm_ps = projp.tile([64, 512], f32, tag="warm", bufs=1)
                for _ in range(15):
                    nc.tensor.matmul(warm_ps[:], wsrc[:, 0:64], wsrc[:],
                                     start=True, stop=True)
                qk_chunk(projp, k0, wk_sb, bk_sb, 0, 128, 0)()
                qk_chunk(projp, q0, wq_sb, bq_sb, 0, 128, 0)()
                qk_chunk(projp, q0, wq_sb, bq_sb, 0, 128, 1)()
                for st in range(6):
                    v_tile(projp, st)()

            # ---- attention: 96-block pipeline with interleaved proj ----
            qk_slices = (
                (q0[0:64, :], k0[0:64, :]),
                (q0[64:128, :], k0[64:128, :]),
                (qk1[0:64, :], k1a[:, :]),
            )

            with tc.tile_pool(name="att", bufs=3, space="PSUM") as attp:
                # filler schedule: block -> one full proj unit (a chunk or a
                # V tile), borrowing a "ps"-tag PSUM slot for its lifetime
                filler = {b: [] for b in range(NBLK)}

                def aux_chunk(dst, wsb, bsb, col0, m, c):
                    return qk_chunk(attp, dst, wsb, bsb, col0, m, c,
                                    tag="ps", bufs=3)

                filler[0].append(aux_chunk(k0, wk_sb, bk_sb, 0, 128, 1))
                filler[3].append(aux_chunk(k0, wk_sb, bk_sb, 0, 128, 2))
                filler[6].append(aux_chunk(k0, wk_sb, bk_sb, 0, 128, 3))
                filler[10].append(aux_chunk(q0, wq_sb, bq_sb, 0, 128, 2))
                filler[13].append(aux_chunk(q0, wq_sb, bq_sb, 0, 128, 3))
                vslots = [1, 2, 4, 5, 7, 8, 9, 11, 12, 14]
                for st, b in zip(range(6, SKT), vslots):
                    filler[b].append(v_tile(attp, st, tag="ps", bufs=3))
                # head-2 [q1;k1] m=128 chunks, then realign k1 to partitions
                # 0-63 with an SBUF->SBUF DMA (off-engine)
                for c in range(4):
                    filler[17 + 6 * c].append(
                        aux_chunk(qk1, wqk1_sb, bqk1_sb, 0, 128, c))
                filler[44].append(
                    lambda: nc.sync.dma_start(k1a[:], qk1[64:128, :]))

                blocks = [(j, sqh, st)
                          for j in range(HPC) for sqh in range(2)
                          for st in range(SKT)]
                pend = []            # PV lag queue: (j, sqh, st, pt, po)
                po_cur = {}
                bt_cur = {}

                def emit_pv(j, sqh, st, pt, po):
                    last = st == SKT - 1
                    ot = otp.tile([D + 1, 1024], f32, name="ot") if last else None
                    for cc in range(2):
                        nc.tensor.matmul(
                            po[:, cc * 512:(cc + 1) * 512],
                            vp[j][:, st * (D + 1):(st + 1) * (D + 1)],
                            pt[:, cc * 512:(cc + 1) * 512],
                            start=(st == 0), stop=last,
                        )
                        if last:
                            nc.vector.tensor_copy(
                                ot[:, cc * 512:(cc + 1) * 512],
                                po[:, cc * 512:(cc + 1) * 512])
                    if last:
                        nc.sync.dma_start(out[j, sqh], ot[:])

                for b, (j, sqh, st) in enumerate(blocks):
                    if st % 4 == 0:
                        bt = biasp.tile([128, 4096], f16, name="bt")
                        nc.sync.dma_start(bt[:], bias_r[j, sqh, st // 4])
                        bt_cur[(j, sqh)] = bt
                    bt = bt_cur[(j, sqh)]
                    boff = (st % 4) * 1024
                    if st == 0:
                        po_cur[(j, sqh)] = attp.tile([D + 1, 1024], f32,
                                                     tag="po", name="po", bufs=1)
                    qap, kap = qk_slices[j]
                    inj = _inj(b, inj_from)
                    ps = attp.tile([128, 1024], f32, tag="ps", name="ps", bufs=3)
                    if inj:
                        for cc in range(2):
                            nc.tensor.matmul(
                                ps[:, cc * 512:(cc + 1) * 512],
                                ident_sb[:],
                                bt[:, boff + cc * 512:boff + (cc + 1) * 512],
                                start=True, stop=False,
                            )
                    for cc in range(2):
                        nc.tensor.matmul(
                            ps[:, cc * 512:(cc + 1) * 512],
                            kap[:, st * 128:(st + 1) * 128],
                            qap[:, sqh * 1024 + cc * 512:sqh * 1024 + (cc + 1) * 512],
                            start=(not inj), stop=True,
                        )
                    for f in filler[b]:
                        f()
                    pt = ptp.tile([128, 1024], f16, name="pt")
                    nc.scalar.activation(pt[:], ps[:], EXP)
                    if not inj:
                        nc.vector.tensor_mul(pt[:], pt[:],
                                             bt[:, boff:boff + 1024])
                    pend.append((j, sqh, st, pt, po_cur[(j, sqh)]))
                    if len(pend) > 3:
                        emit_pv(*pend.pop(0))
                while pend:
                    emit_pv(*pend.pop(0))

    nc.compile()
    return nc


def _get_nc(inj_from=64):
    if inj_from not in _CACHE:
        _CACHE[inj_from] = _build_nc(inj_from)
    return _CACHE[inj_from]


def _make_in_maps(hidden_states, Wqkv_w, Wqkv_b, bias, indices, inj_from=64):
    hidden_states = np.asarray(hidden_states, dtype=np.float32)
    Wqkv_w = np.asarray(Wqkv_w, dtype=np.float32)
    Wqkv_b = np.asarray(Wqkv_b, dtype=np.float32)
    bias = np.asarray(bias, dtype=np.float32)
    indices = np.asarray(indices, dtype=np.int64)

    scale = 1.0 / math.sqrt(D)
    padded = np.zeros((TOTAL, DIM), dtype=np.float32)
    padded[indices] = hidden_states

    Wq, Wk, Wv = Wqkv_w[0:DIM], Wqkv_w[DIM:2 * DIM], Wqkv_w[2 * DIM:3 * DIM]
    bq_full = Wqkv_b[0:DIM] * scale
    bk_full = Wqkv_b[DIM:2 * DIM]
    ident = np.eye(128, dtype=np.float16)

    in_maps = []
    for c in range(N_CORES):
        b = c // 4
        h0 = (c % 4) * HPC
        r = slice(h0 * D, (h0 + HPC) * D)
        # bias_r[j, sqh, c4, p, st4*1024 + q] =
        #   f(bias[b, h0+j, sqh*1024 + q, (4*c4+st4)*128 + p])   (transposed)
        bias_c = bias[b, h0:h0 + HPC]                    # (3, sq, sk)
        bt = bias_c.transpose(0, 2, 1)                   # (3, sk, sq)
        bt = bt.reshape(HPC, 4, 4, 128, 2, 1024)         # (j, c4, st4, p, sqh, q)
        bt = bt.transpose(0, 4, 1, 3, 2, 5)              # (j, sqh, c4, p, st4, q)
        blk = (np.arange(HPC)[:, None, None, None] * 2 * SKT
               + np.arange(2)[None, :, None, None] * SKT
               + np.arange(4)[None, None, :, None] * 4
               + np.arange(4)[None, None, None, :])      # (j, sqh, c4, st4)
        expb = blk < inj_from
        bt = np.where(expb[:, :, :, None, :, None], np.exp(bt), bt)
        bias_r = np.ascontiguousarray(
            bt.reshape(HPC, 2, 4, 128, 4096).astype(np.float16))
        # pack k/q weight k-tiles + hT cols 0-511 into the "early" bundle:
        # early[p, (w*6+i)*192+c] = W.T[i*128+p, c];
        # early[p, 2304+i*512+c]  = hT[i*128+p, c]  (c < 512)
        hTc = padded[b * S:(b + 1) * S].T.astype(np.float16)  # (768, 2048)
        wt = np.stack([Wk[r].T, (Wq[r].T * np.float32(scale))])
        wkq = (wt.reshape(2, KT, 128, HPC * D).transpose(2, 0, 1, 3)
               .reshape(128, 2 * KT * HPC * D).astype(np.float16))
        ht0 = (hTc[:, 0:512].reshape(KT, 128, 512).transpose(1, 0, 2)
               .reshape(128, KT * 512))
        early = np.ascontiguousarray(np.concatenate([wkq, ht0], axis=1))
        wv_t = (Wv[r].T.reshape(KT, 128, HPC * D).transpose(1, 0, 2)
                .reshape(128, KT * HPC * D).astype(np.float16))
        # head-2 packed tiles: [wq1(64) | wk1(64)] per k-tile
        wq1 = (Wq[r].T * np.float32(scale))[:, 128:192]
        wk1 = Wk[r].T[:, 128:192]
        wqk1 = (np.concatenate([wq1, wk1], axis=1).astype(np.float16)
                .reshape(KT, 128, 128).transpose(1, 0, 2).reshape(128, KT * 128))
        wvp = np.ascontiguousarray(np.concatenate([wv_t, wqk1], axis=1))
        bvec = np.zeros((128, 5), dtype=np.float32)
        bvec[:, 0] = bq_full[r][0:128]
        bvec[0:64, 1] = bq_full[r][128:192]
        bvec[:, 2] = bk_full[r][0:128]
        bvec[0:64, 3] = bk_full[r][128:192]
        bvec[0:64, 4] = bq_full[r][128:192]
        bvec[64:128, 4] = bk_full[r][128:192]
        in_maps.append({
            "hT": hTc,
            "early": early,
            "wvp": wvp,
            "bvec": bvec,
            "bias_r": bias_r,
            "ident": ident,
        })
    return in_maps


def _assemble(results, Wqkv_b, indices):
    Wqkv_b = np.asarray(Wqkv_b, dtype=np.float32)
    indices = np.asarray(indices, dtype=np.int64)
    bv = Wqkv_b[2 * DIM:3 * DIM]
    out_full = np.empty((TOTAL, DIM), dtype=np.float32)
    for c in range(N_CORES):
        b = c // 4
        h0 = (c % 4) * HPC
        o = np.asarray(results[c]["out"], dtype=np.float32)  # (3, 2, 65, 1024)
        for j in range(HPC):
            h = h0 + j
            oj = np.concatenate([o[j, 0], o[j, 1]], axis=1)  # (65, 2048)
            att = (oj[:D] / oj[D]).T + bv[h * D:(h + 1) * D]
            out_full[b * S:(b + 1) * S, h * D:(h + 1) * D] = att
    return out_full[indices]


INJ_FROM = 96


def kernel(hidden_states, Wqkv_w, Wqkv_b, bias, slopes, cu_seqlens, indices,
           attn_mask, max_seqlen, **_unused):
    from concourse.bass_utils import run_bass_kernel_spmd

    nc = _get_nc(INJ_FROM)
    in_maps = _make_in_maps(hidden_states, Wqkv_w, Wqkv_b, bias, indices,
                            INJ_FROM)
    res = run_bass_kernel_spmd(nc, in_maps, list(range(N_CORES)))
    return _assemble(res.results, Wqkv_b, indices)
